# revision 1
# baseline (speedup 1.0000x reference)
"""GAT (3-layer, PyG-style) Trainium2 Bass kernel, 8-core SPMD.

Sharding: nodes are padded 10000->10240 and split 1280 per core (10 tiles of
128 dst nodes).  Edges (incl. self loops) are routed to the core/tile owning
their destination, padded to a uniform C_max chunks of 128 edges per
(core, tile).  Per layer: sharded dense matmuls build a per-node gather-table
row [xw bf16 | a_src fp32]; the table is AllGather'd; each dst tile gathers
its edges' source rows with one dma_gather, computes per-edge
p = exp(leakyrelu(a_src+a_dst)) (a_dst via a small PE matmul against a
host-streamed transposed indicator), scales messages in place (bf16) and
aggregates them plus the softmax denominator with indicator matmuls into
PSUM.  Softmax max-subtraction is skipped (logits are O(1); the denominator
normalization happens once per node after aggregation).
"""
import os
import numpy as np
import ml_dtypes

import concourse.bacc as bacc
import concourse.tile as tile
import concourse.mybir as mybir
from concourse import library_config
from concourse.bass_utils import run_bass_kernel_spmd

NCORES = 8
N = 10000
NLOC_REAL = 1250          # real nodes per core
NLOC = 1280               # padded nodes per core
NPAD = NLOC * NCORES      # 10240
NTILE = NLOC // 128       # 10
N_FEAT = 256
N_HID = 128
N_HEAD = 4
N_HEAD_LAST = 6
N_CLASS = 40
D01 = N_HID * N_HEAD      # 512
D2 = N_HEAD_LAST * N_CLASS  # 240
NEG = 0.2

ROW01 = 640               # uint16 slots per table row, layers 0/1 (1280 B)
ROW2 = 256                # layer 2 (512 B)

F32 = mybir.dt.float32
BF16 = mybir.dt.bfloat16
U16 = mybir.dt.uint16
I16 = mybir.dt.int16

LAST_RESULTS = None       # test harness can read exec_time_ns etc.
_PROGRAM_CACHE = {}
_NQ4 = os.environ.get("GAT_NQ1", "") == ""


def _pid(n):
    """original node id -> padded global id"""
    return (n // NLOC_REAL) * NLOC + (n % NLOC_REAL)


def _build_program(c_max):
    S = c_max * 128  # edge slots per tile
    nc = bacc.Bacc("TRN2", num_devices=NCORES, debug=False, num_swdge_queues=4)

    # ---------------- kernel I/O ----------------
    xT_in = nc.dram_tensor("xT", [128, 2, NLOC], F32, kind="ExternalInput")
    w0_in = nc.dram_tensor("w0aug", [128, 2, D01 + 8], F32, kind="ExternalInput")
    wskip0_in = nc.dram_tensor("wskip0", [128, 2, D01], F32, kind="ExternalInput")
    w1_in = nc.dram_tensor("w1aug", [128, 4, D01 + 8], F32, kind="ExternalInput")
    w2_in = nc.dram_tensor("w2aug", [128, 4, D2 + 12], F32, kind="ExternalInput")
    wskip2_in = nc.dram_tensor("wskip2", [128, 4, N_CLASS], F32, kind="ExternalInput")
    b0_in = nc.dram_tensor("b0row", [1, D01], F32, kind="ExternalInput")
    b1_in = nc.dram_tensor("b1row", [1, D01], F32, kind="ExternalInput")
    b2_in = nc.dram_tensor("b2row", [1, N_CLASS], F32, kind="ExternalInput")
    ones_in = nc.dram_tensor("ones_row", [1, 128], F32, kind="ExternalInput")
    iota_in = nc.dram_tensor("iota_row", [128, 128], F32, kind="ExternalInput")
    ident_in = nc.dram_tensor("ident", [128, 128], F32, kind="ExternalInput")
    idx_in = nc.dram_tensor("idx_all", [NTILE * 128, S // 16], I16, kind="ExternalInput")
    dstloc_in = nc.dram_tensor("dstloc_all", [NTILE * 128, c_max], F32, kind="ExternalInput")
    indt_in = nc.dram_tensor("indt_all", [NTILE * 128, S], BF16, kind="ExternalInput")
    out_dram = nc.dram_tensor("out_loc", [NLOC, N_CLASS], F32, kind="ExternalOutput")

    tab0_full = nc.dram_tensor("tab0_full", [NPAD, ROW01], U16, addr_space="Shared")
    tab1_full = nc.dram_tensor("tab1_full", [NPAD, ROW01], U16, addr_space="Shared")
    tab2_full = nc.dram_tensor("tab2_full", [NPAD, ROW2], U16, addr_space="Shared")
    RG = [list(range(NCORES))]

    with tile.TileContext(nc, num_cores=NCORES) as tc:
        with (
            tc.tile_pool(name="persist", bufs=1) as pp,
            tc.tile_pool(name="dram", bufs=1, space="DRAM") as dram,
        ):
            nc.gpsimd.load_library(library_config.mlp)

            # ---- resident constants / weights ----
            def load_const(name, ap, shape, dtype=F32):
                t = pp.tile(shape, dtype, tag=name)
                nc.sync.dma_start(t[:], ap)
                return t

            w0_sb = load_const("w0", w0_in[:], [128, 2, D01 + 8])
            wskip0_sb = load_const("wskip0", wskip0_in[:], [128, 2, D01])
            w1_sb = load_const("w1", w1_in[:], [128, 4, D01 + 8])
            w2_sb = load_const("w2", w2_in[:], [128, 4, D2 + 12])
            wskip2_sb = load_const("wskip2", wskip2_in[:], [128, 4, N_CLASS])
            b0row = load_const("b0row", b0_in[:], [1, D01])
            b1row = load_const("b1row", b1_in[:], [1, D01])
            b2row = load_const("b2row", b2_in[:], [1, N_CLASS])
            ones_sb = load_const("ones", ones_in[:], [1, 128])
            iota_sb = load_const("iota", iota_in[:], [128, 1, 128])
            ident_sb = load_const("ident", ident_in[:], [128, 128])

            # bias broadcast rows -> [128, D] via K=1 matmul
            with tc.tile_pool(name="psb", bufs=1, space="PSUM") as psb:
                b_bcast = {}
                for nm, row, d in (("b0", b0row, D01), ("b1", b1row, D01), ("b2", b2row, N_CLASS)):
                    ps = psb.tile([128, d], F32, tag="bias_ps")
                    nc.tensor.matmul(out=ps[:], lhsT=ones_sb[:], rhs=row[:], start=True, stop=True)
                    bb = pp.tile([128, d], F32, tag=f"bb_{nm}")
                    nc.vector.tensor_copy(out=bb[:], in_=ps[:])
                    b_bcast[nm] = bb

            # per-node per-layer a_dst (hi/lo bf16): [128, NTILE, 2H]
            def tset(nm, shape, dtype):
                return [pp.tile(shape, dtype, tag=f"{nm}_{t}", name=f"{nm}_{t}")
                        for t in range(NTILE)]
            adst0_sb = tset("adst0", [128, 8], BF16)
            adst1_sb = tset("adst1", [128, 8], BF16)
            adst2_sb = tset("adst2", [128, 12], BF16)
            skip0_own = tset("skip0", [128, D01], F32)
            x1_own = tset("x1own", [128, D01], F32)
            skip2_own = tset("skip2", [128, N_CLASS], F32)

            tab0_bounce = dram.tile([NLOC, ROW01], U16, tag="tb0")
            tab1_bounce = dram.tile([NLOC, ROW01], U16, tag="tb1")
            tab2_bounce = dram.tile([NLOC, ROW2], U16, tag="tb2")

            # =========================================================
            # dense phase: build table slab rows for layer `lay`
            # =========================================================
            def dense_phase(lay, lhsT_fn, kchunks, waug_sb, dcols, nattn, adst_sb,
                            bounce, skip_w_sb=None, skip_dst=None, skip_cols=0):
                with (
                    tc.tile_pool(name=f"d{lay}", bufs=2, space="PSUM") as pd,
                    tc.tile_pool(name=f"da{lay}", bufs=2, space="PSUM") as pa,
                    tc.tile_pool(name=f"ds{lay}", bufs=2, space="PSUM") as psk,
                    tc.tile_pool(name=f"dsl{lay}", bufs=2) as slab_pool,
                ):
                    for t in range(NTILE):
                        ps_d = pd.tile([128, dcols], F32, tag="ps_d")
                        ps_a = pa.tile([128, 2 * nattn], F32, tag="ps_a")
                        ps_s = (psk.tile([128, max(skip_cols, 1)], F32, tag="ps_s", name="ps_s")
                                if skip_w_sb is not None else None)
                        for k in range(kchunks):
                            lhsT = lhsT_fn(t, k)
                            nc.tensor.matmul(out=ps_d[:], lhsT=lhsT, rhs=waug_sb[:, k, 0:dcols],
                                             start=(k == 0), stop=(k == kchunks - 1))
                            nc.tensor.matmul(out=ps_a[:], lhsT=lhsT,
                                             rhs=waug_sb[:, k, dcols:dcols + 2 * nattn],
                                             start=(k == 0), stop=(k == kchunks - 1))
                            if skip_w_sb is not None:
                                nc.tensor.matmul(out=ps_s[:], lhsT=lhsT, rhs=skip_w_sb[:, k, 0:skip_cols],
                                                 start=(k == 0), stop=(k == kchunks - 1))
                        rowlen = ROW01 if dcols == D01 else ROW2
                        slab = slab_pool.tile([128, rowlen], U16, tag="slab")
                        used = dcols + 2 * nattn
                        nc.vector.memset(slab[:, used:rowlen], 0)
                        # xw -> bf16 payload
                        nc.vector.tensor_copy(out=slab[:, 0:dcols].bitcast(BF16), in_=ps_d[:])
                        # a_src fp32 raw into slots [dcols : dcols+2*nattn]
                        nc.vector.tensor_copy(out=slab[:, dcols:dcols + 2 * nattn].bitcast(F32),
                                              in_=ps_a[:, 0:nattn])
                        # zero the pad slots once (they are never read; keep them defined)
                        # a_dst hi/lo bf16
                        nc.vector.tensor_copy(out=adst_sb[t][:, 0:nattn], in_=ps_a[:, nattn:2 * nattn])
                        nc.vector.tensor_tensor(out=adst_sb[t][:, nattn:2 * nattn],
                                                in0=ps_a[:, nattn:2 * nattn],
                                                in1=adst_sb[t][:, 0:nattn],
                                                op=mybir.AluOpType.subtract)
                        if skip_w_sb is not None:
                            nc.vector.tensor_copy(out=skip_dst[t][:, 0:skip_cols], in_=ps_s[:])
                        nc.sync.dma_start(bounce[t * 128:(t + 1) * 128, :], slab[:])

            # =========================================================
            # edge phase for layer `lay`
            # =========================================================
            def edge_phase(lay, tab_full, rowlen, dcols, nh, ch, adst_sb, epilogue):
                with (
                    tc.tile_pool(name=f"eg{lay}", bufs=2) as gp,
                    tc.tile_pool(name=f"ei{lay}", bufs=2) as ip,
                    tc.tile_pool(name=f"ea{lay}", bufs=3, space="PSUM") as pagg,
                    tc.tile_pool(name=f"et{lay}", bufs=2, space="PSUM") as padst,
                    tc.tile_pool(name=f"ep{lay}", bufs=2, space="PSUM") as ptr,
                    tc.tile_pool(name=f"ee{lay}", bufs=2) as ep,
                ):
                    for t in range(NTILE):
                        rows = slice(t * 128, (t + 1) * 128)
                        idxs = ip.tile([128, S // 16], I16, tag="idx")
                        nc.sync.dma_start(idxs[:], idx_in[rows, :])
                        dstloc = ip.tile([128, c_max, 1], F32, tag="dstloc")
                        nc.sync.dma_start(dstloc[:], dstloc_in[rows, :])
                        indt = ip.tile([128, S], BF16, tag="indt")
                        nc.sync.dma_start(indt[:], indt_in[rows, :])
                        gath = gp.tile([128, c_max, rowlen], U16, tag="gath")
                        ngr = (c_max + 7) // 8  # <=1024 idxs per gather (larger hangs HW)
                        bounds = [round(i * c_max / ngr) for i in range(ngr + 1)]
                        for gi in range(ngr):
                            g0, g1 = bounds[gi], bounds[gi + 1]
                            nidx = (g1 - g0) * 128
                            nc.gpsimd.dma_gather(
                                out_ap=gath[:, g0:g1, :], in_ap=tab_full[:],
                                idxs_ap=idxs[:, g0 * 8:g1 * 8],
                                num_idxs=nidx, num_idxs_reg=nidx, elem_size=rowlen,
                                queue_num=(t * 3 + gi) % 4 if _NQ4 else 0)

                        # indicator [e, n] for every chunk in one op
                        ind = ip.tile([128, c_max, 128], BF16, tag="ind")
                        nc.vector.tensor_tensor(
                            out=ind[:],
                            in0=dstloc[:].to_broadcast([128, c_max, 128]),
                            in1=iota_sb[:].to_broadcast([128, c_max, 128]),
                            op=mybir.AluOpType.is_equal)

                        # a_dst per edge: one small matmul per chunk
                        ps_adst = padst.tile([128, c_max + 1, 2 * nh], F32, tag="ps_adst", name="ps_adst")
                        for c in range(c_max):
                            nc.tensor.matmul(out=ps_adst[:, c, :],
                                             lhsT=indt[:, c * 128:(c + 1) * 128],
                                             rhs=adst_sb[t][:],
                                             start=True, stop=True)

                        # s = a_src + a_dst ; p = exp(leakyrelu(s))
                        asrc = gath[:, :, dcols:dcols + 2 * nh].bitcast(F32)  # [128, c_max, nh]
                        s_all = ep.tile([128, c_max, nh], F32, tag="s")
                        nc.vector.tensor_tensor(out=s_all[:], in0=ps_adst[:, 0:c_max, 0:nh],
                                                in1=asrc, op=mybir.AluOpType.add)
                        nc.vector.tensor_tensor(out=s_all[:], in0=s_all[:],
                                                in1=ps_adst[:, 0:c_max, nh:2 * nh],
                                                op=mybir.AluOpType.add)
                        s02 = ep.tile([128, c_max, nh], F32, tag="s02")
                        nc.vector.tensor_scalar_mul(out=s02[:], in0=s_all[:], scalar1=NEG)
                        nc.vector.tensor_tensor(out=s_all[:], in0=s_all[:], in1=s02[:],
                                                op=mybir.AluOpType.max)
                        p_bf = ep.tile([128, c_max, nh, 1], BF16, tag="pbf")
                        nc.scalar.activation(p_bf[:, :, :, 0], s_all[:],
                                             mybir.ActivationFunctionType.Exp)

                        # messages scaled in place (bf16), one op per head
                        for h in range(nh):
                            mh = gath[:, :, h * ch:(h + 1) * ch].bitcast(BF16)
                            nc.vector.tensor_tensor(
                                out=mh, in0=mh,
                                in1=p_bf[:, :, h, :].to_broadcast([128, c_max, ch]),
                                op=mybir.AluOpType.mult)

                        # aggregate messages + denominators
                        ps_agg = pagg.tile([128, dcols], F32, tag="ps_agg")
                        ps_den = ps_adst[:, c_max, 0:nh]
                        for c in range(c_max):
                            lhsT = ind[:, c, :]
                            nc.tensor.matmul(out=ps_agg[:], lhsT=lhsT,
                                             rhs=gath[:, c, 0:dcols].bitcast(BF16),
                                             start=(c == 0), stop=(c == c_max - 1))
                            nc.tensor.matmul(out=ps_den, lhsT=lhsT,
                                             rhs=p_bf[:, c, :, 0],
                                             start=(c == 0), stop=(c == c_max - 1))
                        epilogue(t, ps_agg, ps_den, ep, ptr)

            # =========================================================
            # layer 0
            # =========================================================
            xT0_sb = pp.tile([128, 2, NLOC], F32, tag="xT0")
            nc.sync.dma_start(xT0_sb[:], xT_in[:])
            dense_phase(0, lambda t, k: xT0_sb[:, k, t * 128:(t + 1) * 128], 2, w0_sb, D01, N_HEAD, adst0_sb, tab0_bounce,
                        skip_w_sb=wskip0_sb, skip_dst=skip0_own, skip_cols=D01)
            nc.gpsimd.collective_compute("AllGather", mybir.AluOpType.bypass,
                                         replica_groups=RG,
                                         ins=[tab0_bounce[:].opt()], outs=[tab0_full[:].opt()])

            x1T_sb = tset("x1T", [128, 4, 128], F32)
            x2T_sb = tset("x2T", [128, 4, 128], F32)

            def make_next_x(t, ps_agg, ps_den, ep, ptr, nh, skip_src, b_bc, xout_own, xT_next):
                """v = ps_agg/den + skip + b ; x = elu(v); write x (+transpose)."""
                recip = ep.tile([128, nh], F32, tag="recip")
                nc.vector.reciprocal(out=recip[:], in_=ps_den[:])
                v = ep.tile([128, D01], F32, tag="v")
                for h in range(N_HEAD):
                    nc.vector.tensor_scalar_mul(out=v[:, h * 128:(h + 1) * 128],
                                                in0=ps_agg[:, h * 128:(h + 1) * 128],
                                                scalar1=recip[:, h:h + 1])
                nc.vector.tensor_tensor(out=v[:], in0=v[:], in1=skip_src[t][:],
                                        op=mybir.AluOpType.add)
                nc.vector.tensor_tensor(out=v[:], in0=v[:], in1=b_bc[:],
                                        op=mybir.AluOpType.add)
                # elu(v) = max(v,0) - 1 + exp(min(v,0))
                vneg = ep.tile([128, D01], F32, tag="vneg")
                nc.vector.tensor_scalar_min(out=vneg[:], in0=v[:], scalar1=0.0)
                eneg = ep.tile([128, D01], F32, tag="eneg")
                nc.scalar.activation(eneg[:], vneg[:], mybir.ActivationFunctionType.Exp)
                x = xout_own[t][:]
                nc.vector.tensor_scalar(out=v[:], in0=v[:], scalar1=0.0, scalar2=-1.0,
                                        op0=mybir.AluOpType.max, op1=mybir.AluOpType.add)
                nc.vector.tensor_tensor(out=x, in0=v[:], in1=eneg[:], op=mybir.AluOpType.add)
                # transpose x tile into xT_next
                for j in range(4):
                    ps_t = ptr.tile([128, 128], F32, tag="ps_t")
                    nc.tensor.transpose(out=ps_t[:], in_=xout_own[t][:, j * 128:(j + 1) * 128],
                                        identity=ident_sb[:])
                    nc.vector.tensor_copy(out=xT_next[t][:, j, :], in_=ps_t[:])

            edge_phase(0, tab0_full, ROW01, D01, N_HEAD, N_HID, adst0_sb,
                       lambda t, pa, pd, ep, ptr: make_next_x(
                           t, pa, pd, ep, ptr, N_HEAD, skip0_own, b_bcast["b0"], x1_own, x1T_sb))

            # =========================================================
            # layer 1
            # =========================================================
            dense_phase(1, lambda t, k: x1T_sb[t][:, k, :], 4, w1_sb, D01, N_HEAD, adst1_sb, tab1_bounce)
            nc.gpsimd.collective_compute("AllGather", mybir.AluOpType.bypass,
                                         replica_groups=RG,
                                         ins=[tab1_bounce[:].opt()], outs=[tab1_full[:].opt()])
            x2_own = skip0_own  # reuse slot (skip0 dead after layer-0 edge phase)
            edge_phase(1, tab1_full, ROW01, D01, N_HEAD, N_HID, adst1_sb,
                       lambda t, pa, pd, ep, ptr: make_next_x(
                           t, pa, pd, ep, ptr, N_HEAD, x1_own, b_bcast["b1"], x2_own, x2T_sb))

            # =========================================================
            # layer 2
            # =========================================================
            dense_phase(2, lambda t, k: x2T_sb[t][:, k, :], 4, w2_sb, D2, N_HEAD_LAST, adst2_sb, tab2_bounce,
                        skip_w_sb=wskip2_sb, skip_dst=skip2_own, skip_cols=N_CLASS)
            nc.gpsimd.collective_compute("AllGather", mybir.AluOpType.bypass,
                                         replica_groups=RG,
                                         ins=[tab2_bounce[:].opt()], outs=[tab2_full[:].opt()])

            def final_epilogue(t, ps_agg, ps_den, ep, ptr):
                recip = ep.tile([128, N_HEAD_LAST], F32, tag="recip2")
                nc.vector.reciprocal(out=recip[:], in_=ps_den[:])
                nc.vector.tensor_scalar_mul(out=recip[:], in0=recip[:], scalar1=1.0 / N_HEAD_LAST)
                acc = ep.tile([128, N_CLASS], F32, tag="acc")
                tmp = ep.tile([128, N_CLASS], F32, tag="tmp")
                for h in range(N_HEAD_LAST):
                    dst = acc if h == 0 else tmp
                    nc.vector.tensor_scalar_mul(out=dst[:],
                                                in0=ps_agg[:, h * N_CLASS:(h + 1) * N_CLASS],
                                                scalar1=recip[:, h:h + 1])
                    if h > 0:
                        nc.vector.tensor_tensor(out=acc[:], in0=acc[:], in1=tmp[:],
                                                op=mybir.AluOpType.add)
                nc.vector.tensor_tensor(out=acc[:], in0=acc[:], in1=skip2_own[t][:],
                                        op=mybir.AluOpType.add)
                nc.vector.tensor_tensor(out=acc[:], in0=acc[:], in1=b_bcast["b2"][:],
                                        op=mybir.AluOpType.add)
                nc.sync.dma_start(out_dram[t * 128:(t + 1) * 128, :], acc[:])

            edge_phase(2, tab2_full, ROW2, D2, N_HEAD_LAST, N_CLASS, adst2_sb, final_epilogue)

    nc.compile()
    return nc


def _prep_inputs(x, edge_index, W0, a_src0, a_dst0, b0, Wskip_in,
                 W1, a_src1, a_dst1, b1, W2, a_src2, a_dst2, b2, Wskip_out):
    """Host-side routing/layout (no network FLOPs besides weight folding)."""
    x = np.asarray(x, dtype=np.float32)
    ei = np.asarray(edge_index)
    loops = np.arange(N, dtype=np.int64)
    src = np.concatenate([ei[0], loops]).astype(np.int64)
    dst = np.concatenate([ei[1], loops]).astype(np.int64)

    # fold attention vectors into the weight matrices:
    # w_asrc[i, h] = sum_c W[i, h*ch + c] * a_src[h, c]
    def fold(W, a_s, a_d, heads, ch):
        Wr = np.asarray(W, np.float32).reshape(-1, heads, ch)
        ws = np.einsum("ihc,hc->ih", Wr, np.asarray(a_s, np.float32))
        wd = np.einsum("ihc,hc->ih", Wr, np.asarray(a_d, np.float32))
        return np.concatenate([np.asarray(W, np.float32), ws, wd], axis=1)

    w0aug = fold(W0, a_src0, a_dst0, N_HEAD, N_HID)        # [256, 520]
    w1aug = fold(W1, a_src1, a_dst1, N_HEAD, N_HID)        # [512, 520]
    w2aug = fold(W2, a_src2, a_dst2, N_HEAD_LAST, N_CLASS)  # [512, 252]

    # ---- edge routing ----
    core = dst // NLOC_REAL
    loc = dst - core * NLOC_REAL          # 0..1249
    tile_id = loc // 128
    dst_local = loc - tile_id * 128
    src_pid = _pid(src).astype(np.int64)

    counts = np.zeros((NCORES, NTILE), dtype=np.int64)
    np.add.at(counts, (core, tile_id), 1)
    c_max = int(np.ceil(counts.max() / 128))
    S = c_max * 128

    # slot assignment per (core, tile)
    order = np.lexsort((tile_id, core))
    src_s, dl_s, core_s, tile_s = src_pid[order], dst_local[order], core[order], tile_id[order]
    idx_all = np.zeros((NCORES, NTILE, 128, S // 16), dtype=np.int16)
    dstloc_all = np.full((NCORES, NTILE, 128, c_max), -1.0, dtype=np.float32)
    indt_all = np.zeros((NCORES, NTILE, 128, S), dtype=ml_dtypes.bfloat16)
    pos = 0
    for k in range(NCORES):
        for t in range(NTILE):
            cnt = counts[k, t]
            sp = src_s[pos:pos + cnt]
            dl = dl_s[pos:pos + cnt]
            assert (core_s[pos:pos + cnt] == k).all() and (tile_s[pos:pos + cnt] == t).all()
            pos += cnt
            slots_src = np.zeros(S, dtype=np.int16)
            slots_src[:cnt] = sp.astype(np.int16)
            slots_dl = np.full(S, -1.0, dtype=np.float32)
            slots_dl[:cnt] = dl.astype(np.float32)
            if t == NTILE - 1:
                # give padded nodes (local 98..127 of the last tile) pad edges so
                # their softmax denominators stay finite (rows are dropped later)
                npads = S - cnt
                if npads > 0:
                    padnodes = 98 + (np.arange(npads) % 30)
                    slots_dl[cnt:] = padnodes.astype(np.float32)
            j = np.arange(S)
            idx_wrapped = np.zeros((16, S // 16), dtype=np.int16)
            idx_wrapped[j % 16, j // 16] = slots_src
            idx_all[k, t] = np.tile(idx_wrapped, (8, 1))
            dstloc_all[k, t, j % 128, j // 128] = slots_dl
            # IndT[n, c*128 + e] = (dst_local of slot (c,e)) == n
            sl2 = slots_dl.reshape(c_max, 128)           # [c, e]
            m = (sl2[None, :, :] == np.arange(128, dtype=np.float32)[:, None, None])
            indt_all[k, t] = m.reshape(128, S).astype(ml_dtypes.bfloat16)

    # ---- x transpose per core: [128, 2, 1280] ----
    xpad = np.zeros((NPAD, N_FEAT), dtype=np.float32)
    xpad[_pid(np.arange(N))] = x
    xT = np.zeros((NCORES, 128, 2, NLOC), dtype=np.float32)
    for k in range(NCORES):
        xl = xpad[k * NLOC:(k + 1) * NLOC]               # [1280, 256]
        xT[k] = xl.T.reshape(2, 128, NLOC).transpose(1, 0, 2)

    def wlayout(W, kchunks, cols):
        # [in, cols] -> [128, kchunks, cols]
        return np.ascontiguousarray(
            np.asarray(W, np.float32).reshape(kchunks, 128, cols).transpose(1, 0, 2))

    common = {
        "w0aug": wlayout(w0aug, 2, D01 + 8),
        "wskip0": wlayout(np.asarray(Wskip_in, np.float32), 2, D01),
        "w1aug": wlayout(w1aug, 4, D01 + 8),
        "w2aug": wlayout(w2aug, 4, D2 + 12),
        "wskip2": wlayout(np.asarray(Wskip_out, np.float32), 4, N_CLASS),
        "b0row": np.asarray(b0, np.float32).reshape(1, D01),
        "b1row": np.asarray(b1, np.float32).reshape(1, D01),
        "b2row": np.asarray(b2, np.float32).reshape(1, N_CLASS),
        "ones_row": np.ones((1, 128), dtype=np.float32),
        "iota_row": np.tile(np.arange(128, dtype=np.float32), (128, 1)),
        "ident": np.eye(128, dtype=np.float32),
    }
    in_maps = []
    for k in range(NCORES):
        m = dict(common)
        m["xT"] = xT[k]
        m["idx_all"] = idx_all[k].reshape(NTILE * 128, S // 16)
        m["dstloc_all"] = dstloc_all[k].reshape(NTILE * 128, c_max)
        m["indt_all"] = indt_all[k].reshape(NTILE * 128, S)
        in_maps.append(m)
    return c_max, in_maps


def kernel(**inputs):
    global LAST_RESULTS
    c_max, in_maps = _prep_inputs(**inputs)
    if c_max not in _PROGRAM_CACHE:
        _PROGRAM_CACHE[c_max] = _build_program(c_max)
    nc = _PROGRAM_CACHE[c_max]
    import os
    trace = bool(int(os.environ.get("GAT_TRACE", "0")))
    br = run_bass_kernel_spmd(nc, in_maps, list(range(NCORES)), trace=trace)
    LAST_RESULTS = br
    out = np.concatenate([np.asarray(r["out_loc"]) for r in br.results], axis=0)
    out = out.reshape(NCORES, NLOC, N_CLASS)[:, :NLOC_REAL].reshape(-1, N_CLASS)
    return np.ascontiguousarray(out[:N], dtype=np.float32)



# revision 8
# speedup vs baseline: 1.0209x; 1.0209x over previous
"""GAT (3-layer, PyG-style) Trainium2 Bass kernel, 8-core SPMD.

Sharding: nodes are padded 10000->10240 and split 1280 per core (10 tiles of
128 dst nodes).  Edges (incl. self loops) are routed to the core/tile owning
their destination, padded to a uniform C_max chunks of 128 edges per
(core, tile).  Per layer: sharded dense matmuls (bf16) build a per-node
gather-table row [xw bf16 | a_src fp32]; the table is AllGather'd in 2
chunks (overlapping the dense phase); each dst tile gathers its edges'
source rows with dma_gather, computes per-edge p = exp(leakyrelu(a_src +
a_dst)) (a_dst via a small PE matmul against a host-streamed transposed
indicator), scales messages in place (bf16) and aggregates them plus the
softmax denominator with host-streamed indicator matmuls into PSUM.
Softmax max-subtraction is skipped (logits are O(1); the denominator
normalization happens once per node after aggregation).
"""
import os
import numpy as np
import ml_dtypes

import concourse.bacc as bacc
import concourse.bass as bass
import concourse.tile as tile
import concourse.mybir as mybir
from concourse import library_config
from concourse.bass_utils import run_bass_kernel_spmd

NCORES = 8
N = 10000
NLOC_REAL = 1250          # real nodes per core
NLOC = 1280               # padded nodes per core
NPAD = NLOC * NCORES      # 10240
NTILE = NLOC // 128       # 10
CHUNKS = 2                # AllGather chunks per layer
TPC = NTILE // CHUNKS     # tiles per AG chunk
N_FEAT = 256
N_HID = 128
N_HEAD = 4
N_HEAD_LAST = 6
N_CLASS = 40
D01 = N_HID * N_HEAD      # 512
D2 = N_HEAD_LAST * N_CLASS  # 240
NEG = 0.2

ROW01 = 640               # uint16 slots per table row, layers 0/1 (1280 B)
ROW2 = 256                # layer 2 (512 B)
ASRC2 = D2 + N_HEAD_LAST  # 246: layer-2 a_src (bf16) slots start (after p slots)

F32 = mybir.dt.float32
BF16 = mybir.dt.bfloat16
U16 = mybir.dt.uint16
I16 = mybir.dt.int16

LAST_RESULTS = None       # test harness can read exec_time_ns etc.
_PROGRAM_CACHE = {}
_NQ4 = os.environ.get("GAT_NQ1", "") == ""
_INDIRECT = os.environ.get("GAT_INDIRECT", "0") == "1"
I32 = mybir.dt.int32


def _pid(n):
    """original node id -> padded global id"""
    return (n // NLOC_REAL) * NLOC + (n % NLOC_REAL)


def _row_of(pid):
    """padded global id -> table row under the chunked-AG layout"""
    k, loc = pid // NLOC, pid % NLOC
    tt, r = loc // 128, loc % 128
    return (tt // TPC) * (NCORES * TPC * 128) + k * (TPC * 128) + (tt % TPC) * 128 + r


def _build_program(c_max):
    S = c_max * 128  # edge slots per tile
    nc = bacc.Bacc("TRN2", num_devices=NCORES, debug=False, num_swdge_queues=4)

    # ---------------- kernel I/O ----------------
    xT_in = nc.dram_tensor("xT", [128, 2, NLOC], BF16, kind="ExternalInput")
    w0_in = nc.dram_tensor("w0aug", [128, 2, D01 + 8], BF16, kind="ExternalInput")
    wskip0_in = nc.dram_tensor("wskip0", [128, 2, D01], BF16, kind="ExternalInput")
    w1_in = nc.dram_tensor("w1aug", [128, 4, D01 + 8], BF16, kind="ExternalInput")
    w2_in = nc.dram_tensor("w2aug", [128, 4, D2 + 12], BF16, kind="ExternalInput")
    wskip2_in = nc.dram_tensor("wskip2", [128, 4, N_CLASS], BF16, kind="ExternalInput")
    b0_in = nc.dram_tensor("b0row", [1, D01], F32, kind="ExternalInput")
    b1_in = nc.dram_tensor("b1row", [1, D01], F32, kind="ExternalInput")
    b2_in = nc.dram_tensor("b2row", [1, N_CLASS], F32, kind="ExternalInput")
    ones_in = nc.dram_tensor("ones_row", [1, 128], F32, kind="ExternalInput")
    ident_in = nc.dram_tensor("ident", [128, 128], BF16, kind="ExternalInput")
    idx_in = nc.dram_tensor("idx_all", [NTILE * 128, S // 16], I16, kind="ExternalInput")
    ind_in = nc.dram_tensor("ind_all", [NTILE * 128, S], BF16, kind="ExternalInput")
    indt_in = nc.dram_tensor("indt_all", [NTILE * 128, S], BF16, kind="ExternalInput")
    out_dram = nc.dram_tensor("out_loc", [NLOC, N_CLASS], F32, kind="ExternalOutput")

    tab0_full = nc.dram_tensor("tab0_full", [NPAD, ROW01], U16, addr_space="Shared")
    tab1_full = nc.dram_tensor("tab1_full", [NPAD, ROW01], U16, addr_space="Shared")
    tab2_full = nc.dram_tensor("tab2_full", [NPAD, ROW2], U16, addr_space="Shared")
    RG = [list(range(NCORES))]
    CROWS = TPC * 128          # bounce rows per AG chunk
    FROWS = NCORES * CROWS     # full-table rows per AG chunk

    with tile.TileContext(nc, num_cores=NCORES) as tc:
        with (
            tc.tile_pool(name="persist", bufs=1) as pp,
            tc.tile_pool(name="dram", bufs=1, space="DRAM") as dram,
        ):
            nc.gpsimd.load_library(library_config.mlp)

            # ---- resident constants / weights ----
            def load_const(name, ap, shape, dtype=F32):
                t = pp.tile(shape, dtype, tag=name)
                nc.sync.dma_start(t[:], ap)
                return t

            w0_sb = load_const("w0", w0_in[:], [128, 2, D01 + 8], BF16)
            wskip0_sb = load_const("wskip0", wskip0_in[:], [128, 2, D01], BF16)
            w1_sb = load_const("w1", w1_in[:], [128, 4, D01 + 8], BF16)
            w2_sb = load_const("w2", w2_in[:], [128, 4, D2 + 12], BF16)
            wskip2_sb = load_const("wskip2", wskip2_in[:], [128, 4, N_CLASS], BF16)
            b0row = load_const("b0row", b0_in[:], [1, D01])
            b1row = load_const("b1row", b1_in[:], [1, D01])
            b2row = load_const("b2row", b2_in[:], [1, N_CLASS])
            ones_sb = load_const("ones", ones_in[:], [1, 128])
            ident_sb = load_const("ident", ident_in[:], [128, 128], BF16)

            # bias broadcast rows -> [128, D] via K=1 matmul
            with tc.tile_pool(name="psb", bufs=1, space="PSUM") as psb:
                b_bcast = {}
                for nm, row, d in (("b0", b0row, D01), ("b1", b1row, D01), ("b2", b2row, N_CLASS)):
                    ps = psb.tile([128, d], F32, tag="bias_ps")
                    nc.tensor.matmul(out=ps[:], lhsT=ones_sb[:], rhs=row[:], start=True, stop=True)
                    bb = pp.tile([128, d], F32, tag=f"bb_{nm}")
                    nc.vector.tensor_copy(out=bb[:], in_=ps[:])
                    b_bcast[nm] = bb

            # per-node per-layer a_dst (hi/lo bf16): [128, NTILE, 2H]
            def tset(nm, shape, dtype):
                return [pp.tile(shape, dtype, tag=f"{nm}_{t}", name=f"{nm}_{t}")
                        for t in range(NTILE)]
            adst0_sb = tset("adst0", [128, 8], BF16)
            adst1_sb = tset("adst1", [128, 8], BF16)
            adst2_sb = tset("adst2", [128, 12], BF16)
            skip0_own = tset("skip0", [128, D01], F32)
            x1_own = tset("x1own", [128, D01], F32)
            skip2_own = tset("skip2", [128, N_CLASS], F32)

            tab0_bounce = dram.tile([NLOC, ROW01], U16, tag="tb0")
            tab1_bounce = dram.tile([NLOC, ROW01], U16, tag="tb1")
            tab2_bounce = dram.tile([NLOC, ROW2], U16, tag="tb2")

            def ag_chunk(bounce, full, rowlen, j):
                nc.gpsimd.collective_compute(
                    "AllGather", mybir.AluOpType.bypass, replica_groups=RG,
                    ins=[bounce[j * CROWS:(j + 1) * CROWS, :].opt()],
                    outs=[full[j * FROWS:(j + 1) * FROWS, :].opt()])

            # =========================================================
            # dense phase: build table slab rows for layer `lay`
            # =========================================================
            def dense_phase(lay, lhsT_fn, kchunks, waug_sb, dcols, nattn, adst_sb,
                            bounce, full, rowlen, asrc_off,
                            skip_w_sb=None, skip_dst=None, skip_cols=0):
                with (
                    tc.tile_pool(name=f"d{lay}", bufs=2, space="PSUM") as pd,
                    tc.tile_pool(name=f"da{lay}", bufs=2, space="PSUM") as pa,
                    tc.tile_pool(name=f"ds{lay}", bufs=2, space="PSUM") as psk,
                    tc.tile_pool(name=f"dsl{lay}", bufs=2) as slab_pool,
                ):
                    for t in range(NTILE):
                        ps_d = pd.tile([128, dcols], F32, tag="ps_d")
                        ps_a = pa.tile([128, 2 * nattn], F32, tag="ps_a")
                        ps_s = (psk.tile([128, max(skip_cols, 1)], F32, tag="ps_s", name="ps_s")
                                if skip_w_sb is not None else None)
                        for k in range(kchunks):
                            lhsT = lhsT_fn(t, k)
                            nc.tensor.matmul(out=ps_d[:], lhsT=lhsT, rhs=waug_sb[:, k, 0:dcols],
                                             start=(k == 0), stop=(k == kchunks - 1))
                            nc.tensor.matmul(out=ps_a[:], lhsT=lhsT,
                                             rhs=waug_sb[:, k, dcols:dcols + 2 * nattn],
                                             start=(k == 0), stop=(k == kchunks - 1))
                            if skip_w_sb is not None:
                                nc.tensor.matmul(out=ps_s[:], lhsT=lhsT, rhs=skip_w_sb[:, k, 0:skip_cols],
                                                 start=(k == 0), stop=(k == kchunks - 1))
                        asrc_bf16 = asrc_off != dcols  # layer-2 layout
                        slab = slab_pool.tile([128, rowlen], U16, tag="slab")
                        used = asrc_off + (nattn if asrc_bf16 else 2 * nattn)
                        nc.vector.memset(slab[:, used:rowlen], 0)
                        if asrc_bf16:
                            # layer-2 layout has p slots between xw and a_src
                            nc.vector.memset(slab[:, dcols:asrc_off], 0)
                        # xw -> bf16 payload
                        nc.vector.tensor_copy(out=slab[:, 0:dcols].bitcast(BF16), in_=ps_d[:])
                        if asrc_bf16:
                            nc.vector.tensor_copy(out=slab[:, asrc_off:asrc_off + nattn].bitcast(BF16),
                                                  in_=ps_a[:, 0:nattn])
                        else:
                            # a_src fp32 raw
                            nc.vector.tensor_copy(out=slab[:, asrc_off:asrc_off + 2 * nattn].bitcast(F32),
                                                  in_=ps_a[:, 0:nattn])
                        # a_dst hi/lo bf16
                        nc.vector.tensor_copy(out=adst_sb[t][:, 0:nattn], in_=ps_a[:, nattn:2 * nattn])
                        nc.vector.tensor_tensor(out=adst_sb[t][:, nattn:2 * nattn],
                                                in0=ps_a[:, nattn:2 * nattn],
                                                in1=adst_sb[t][:, 0:nattn],
                                                op=mybir.AluOpType.subtract)
                        if skip_w_sb is not None:
                            nc.vector.tensor_copy(out=skip_dst[t][:, 0:skip_cols], in_=ps_s[:])
                        nc.sync.dma_start(bounce[t * 128:(t + 1) * 128, :], slab[:])
                        if (t + 1) % TPC == 0:
                            ag_chunk(bounce, full, rowlen, t // TPC)

            # =========================================================
            # edge phase for layer `lay`
            # =========================================================
            def edge_phase(lay, tab_full, rowlen, dcols, nh, ch, adst_sb, epilogue):
                merge_den = (rowlen == ROW2)  # layer 2: p rides in the payload
                asrc_off = ASRC2 if merge_den else dcols
                acols = dcols + nh if merge_den else dcols
                with (
                    tc.tile_pool(name=f"eg{lay}", bufs=2) as gp,
                    tc.tile_pool(name=f"ei{lay}", bufs=2) as ip,
                    tc.tile_pool(name=f"ea{lay}", bufs=3, space="PSUM") as pagg,
                    tc.tile_pool(name=f"et{lay}", bufs=2, space="PSUM") as padst,
                    tc.tile_pool(name=f"ep{lay}", bufs=2, space="PSUM") as ptr,
                    tc.tile_pool(name=f"ee{lay}", bufs=2) as ep,
                ):
                    for t in range(NTILE):
                        rows = slice(t * 128, (t + 1) * 128)
                        idxs = ip.tile([128, S // 16], I16, tag="idx")
                        nc.sync.dma_start(idxs[:], idx_in[rows, :])
                        ind = ip.tile([128, c_max, 128], BF16, tag="ind")
                        nc.sync.dma_start(ind[:], ind_in[rows, :])
                        indt = ip.tile([128, S], BF16, tag="indt")
                        nc.sync.dma_start(indt[:], indt_in[rows, :])
                        gath = gp.tile([128, c_max, rowlen], U16, tag="gath")
                        ngr = (c_max + 7) // 8  # <=1024 idxs per gather (larger hangs HW)
                        bounds = [round(i * c_max / ngr) for i in range(ngr + 1)]
                        for gi in range(ngr):
                            g0, g1 = bounds[gi], bounds[gi + 1]
                            nidx = (g1 - g0) * 128
                            nc.gpsimd.dma_gather(
                                out_ap=gath[:, g0:g1, :], in_ap=tab_full[:],
                                idxs_ap=idxs[:, g0 * 8:g1 * 8],
                                num_idxs=nidx, num_idxs_reg=nidx, elem_size=rowlen,
                                queue_num=(t * 3 + gi) % 4 if _NQ4 else 0)

                        # a_dst per edge: one small matmul per chunk
                        nden = 0 if merge_den else 1
                        ps_adst = padst.tile([128, c_max + nden, 2 * nh], F32, tag="ps_adst", name="ps_adst")
                        for c in range(c_max):
                            nc.tensor.matmul(out=ps_adst[:, c, :],
                                             lhsT=indt[:, c * 128:(c + 1) * 128],
                                             rhs=adst_sb[t][:],
                                             start=True, stop=True)

                        # s = a_src + a_dst ; p = exp(leakyrelu(s))
                        if merge_den:
                            asrc = gath[:, :, asrc_off:asrc_off + nh].bitcast(BF16)
                        else:
                            asrc = gath[:, :, asrc_off:asrc_off + 2 * nh].bitcast(F32)
                        s_all = ep.tile([128, c_max, nh], F32, tag="s")
                        nc.vector.tensor_tensor(out=s_all[:], in0=ps_adst[:, 0:c_max, 0:nh],
                                                in1=asrc, op=mybir.AluOpType.add)
                        nc.vector.tensor_tensor(out=s_all[:], in0=s_all[:],
                                                in1=ps_adst[:, 0:c_max, nh:2 * nh],
                                                op=mybir.AluOpType.add)
                        s02 = ep.tile([128, c_max, nh], F32, tag="s02")
                        nc.vector.tensor_scalar_mul(out=s02[:], in0=s_all[:], scalar1=NEG)
                        nc.vector.tensor_tensor(out=s_all[:], in0=s_all[:], in1=s02[:],
                                                op=mybir.AluOpType.max)
                        if merge_den:
                            # p -> payload slots [dcols:dcols+nh] (bf16)
                            nc.scalar.activation(gath[:, :, dcols:dcols + nh].bitcast(BF16),
                                                 s_all[:],
                                                 mybir.ActivationFunctionType.Exp)
                            p_head = lambda h: gath[:, :, dcols + h:dcols + h + 1].bitcast(BF16)
                            p_chunk = lambda c: gath[:, c, dcols:dcols + nh].bitcast(BF16)
                        else:
                            p_bf = ep.tile([128, c_max, nh, 1], BF16, tag="pbf")
                            nc.scalar.activation(p_bf[:, :, :, 0], s_all[:],
                                                 mybir.ActivationFunctionType.Exp)
                            p_head = lambda h: p_bf[:, :, h, :]
                            p_chunk = lambda c: p_bf[:, c, :, 0]

                        # messages scaled in place (bf16), one op per head
                        for h in range(nh):
                            mh = gath[:, :, h * ch:(h + 1) * ch].bitcast(BF16)
                            nc.vector.tensor_tensor(
                                out=mh, in0=mh,
                                in1=p_head(h).to_broadcast([128, c_max, ch]),
                                op=mybir.AluOpType.mult)

                        # aggregate messages (+ denominators)
                        ps_agg = pagg.tile([128, acols], F32, tag="ps_agg")
                        ps_den = None if merge_den else ps_adst[:, c_max, 0:nh]
                        for c in range(c_max):
                            lhsT = ind[:, c, :]
                            nc.tensor.matmul(out=ps_agg[:], lhsT=lhsT,
                                             rhs=gath[:, c, 0:acols].bitcast(BF16),
                                             start=(c == 0), stop=(c == c_max - 1))
                            if not merge_den:
                                nc.tensor.matmul(out=ps_den, lhsT=lhsT,
                                                 rhs=p_chunk(c),
                                                 start=(c == 0), stop=(c == c_max - 1))
                        if merge_den:
                            ps_den = ps_agg[:, dcols:dcols + nh]
                        epilogue(t, ps_agg, ps_den, ep, ptr)

            # =========================================================
            # layer 0
            # =========================================================
            xT0_sb = pp.tile([128, 2, NLOC], BF16, tag="xT0")
            nc.sync.dma_start(xT0_sb[:], xT_in[:])
            dense_phase(0, lambda t, k: xT0_sb[:, k, t * 128:(t + 1) * 128], 2, w0_sb,
                        D01, N_HEAD, adst0_sb, tab0_bounce, tab0_full, ROW01, D01,
                        skip_w_sb=wskip0_sb, skip_dst=skip0_own, skip_cols=D01)

            x1T_sb = tset("x1T", [128, 4, 128], BF16)
            x2T_sb = tset("x2T", [128, 4, 128], BF16)

            def make_next_x(t, ps_agg, ps_den, ep, ptr, nh, skip_src, b_bc, xout_own, xT_next):
                """v = ps_agg/den + skip + b ; x = elu(v); write x (+transpose)."""
                recip = ep.tile([128, nh], F32, tag="recip")
                nc.vector.reciprocal(out=recip[:], in_=ps_den[:])
                v = ep.tile([128, D01], F32, tag="v")
                for h in range(N_HEAD):
                    nc.vector.tensor_scalar_mul(out=v[:, h * 128:(h + 1) * 128],
                                                in0=ps_agg[:, h * 128:(h + 1) * 128],
                                                scalar1=recip[:, h:h + 1])
                nc.vector.tensor_tensor(out=v[:], in0=v[:], in1=skip_src[t][:],
                                        op=mybir.AluOpType.add)
                nc.vector.tensor_tensor(out=v[:], in0=v[:], in1=b_bc[:],
                                        op=mybir.AluOpType.add)
                # elu(v) = max(v,0) - 1 + exp(min(v,0))
                vneg = ep.tile([128, D01], F32, tag="vneg")
                nc.vector.tensor_scalar_min(out=vneg[:], in0=v[:], scalar1=0.0)
                eneg = ep.tile([128, D01], F32, tag="eneg")
                nc.scalar.activation(eneg[:], vneg[:], mybir.ActivationFunctionType.Exp)
                x = xout_own[t][:]
                nc.vector.tensor_scalar(out=v[:], in0=v[:], scalar1=0.0, scalar2=-1.0,
                                        op0=mybir.AluOpType.max, op1=mybir.AluOpType.add)
                nc.vector.tensor_tensor(out=x, in0=v[:], in1=eneg[:], op=mybir.AluOpType.add)
                # bf16 copy for the transposes / next dense lhsT
                xb = ep.tile([128, D01], BF16, tag="xb")
                nc.vector.tensor_copy(out=xb[:], in_=x)
                for j in range(4):
                    ps_t = ptr.tile([128, 128], BF16, tag="ps_t")
                    nc.tensor.transpose(out=ps_t[:], in_=xb[:, j * 128:(j + 1) * 128],
                                        identity=ident_sb[:])
                    nc.vector.tensor_copy(out=xT_next[t][:, j, :], in_=ps_t[:])

            edge_phase(0, tab0_full, ROW01, D01, N_HEAD, N_HID, adst0_sb,
                       lambda t, pa, pd, ep, ptr: make_next_x(
                           t, pa, pd, ep, ptr, N_HEAD, skip0_own, b_bcast["b0"], x1_own, x1T_sb))

            # =========================================================
            # layer 1
            # =========================================================
            dense_phase(1, lambda t, k: x1T_sb[t][:, k, :], 4, w1_sb,
                        D01, N_HEAD, adst1_sb, tab1_bounce, tab1_full, ROW01, D01)
            x2_own = skip0_own  # reuse slot (skip0 dead after layer-0 edge phase)
            edge_phase(1, tab1_full, ROW01, D01, N_HEAD, N_HID, adst1_sb,
                       lambda t, pa, pd, ep, ptr: make_next_x(
                           t, pa, pd, ep, ptr, N_HEAD, x1_own, b_bcast["b1"], x2_own, x2T_sb))

            # =========================================================
            # layer 2
            # =========================================================
            dense_phase(2, lambda t, k: x2T_sb[t][:, k, :], 4, w2_sb,
                        D2, N_HEAD_LAST, adst2_sb, tab2_bounce, tab2_full, ROW2, ASRC2,
                        skip_w_sb=wskip2_sb, skip_dst=skip2_own, skip_cols=N_CLASS)

            def final_epilogue(t, ps_agg, ps_den, ep, ptr):
                recip = ep.tile([128, N_HEAD_LAST], F32, tag="recip2")
                nc.vector.reciprocal(out=recip[:], in_=ps_den[:])
                nc.vector.tensor_scalar_mul(out=recip[:], in0=recip[:], scalar1=1.0 / N_HEAD_LAST)
                acc = ep.tile([128, N_CLASS], F32, tag="acc")
                tmp = ep.tile([128, N_CLASS], F32, tag="tmp")
                for h in range(N_HEAD_LAST):
                    dst = acc if h == 0 else tmp
                    nc.vector.tensor_scalar_mul(out=dst[:],
                                                in0=ps_agg[:, h * N_CLASS:(h + 1) * N_CLASS],
                                                scalar1=recip[:, h:h + 1])
                    if h > 0:
                        nc.vector.tensor_tensor(out=acc[:], in0=acc[:], in1=tmp[:],
                                                op=mybir.AluOpType.add)
                nc.vector.tensor_tensor(out=acc[:], in0=acc[:], in1=skip2_own[t][:],
                                        op=mybir.AluOpType.add)
                nc.vector.tensor_tensor(out=acc[:], in0=acc[:], in1=b_bcast["b2"][:],
                                        op=mybir.AluOpType.add)
                nc.sync.dma_start(out_dram[t * 128:(t + 1) * 128, :], acc[:])

            edge_phase(2, tab2_full, ROW2, D2, N_HEAD_LAST, N_CLASS, adst2_sb, final_epilogue)

    nc.compile()
    return nc


def _prep_inputs(x, edge_index, W0, a_src0, a_dst0, b0, Wskip_in,
                 W1, a_src1, a_dst1, b1, W2, a_src2, a_dst2, b2, Wskip_out):
    """Host-side routing/layout (no network FLOPs besides weight folding)."""
    x = np.asarray(x, dtype=np.float32)
    ei = np.asarray(edge_index)
    loops = np.arange(N, dtype=np.int64)
    src = np.concatenate([ei[0], loops]).astype(np.int64)
    dst = np.concatenate([ei[1], loops]).astype(np.int64)

    # fold attention vectors into the weight matrices:
    # w_asrc[i, h] = sum_c W[i, h*ch + c] * a_src[h, c]
    def fold(W, a_s, a_d, heads, ch):
        Wr = np.asarray(W, np.float32).reshape(-1, heads, ch)
        ws = np.einsum("ihc,hc->ih", Wr, np.asarray(a_s, np.float32))
        wd = np.einsum("ihc,hc->ih", Wr, np.asarray(a_d, np.float32))
        return np.concatenate([np.asarray(W, np.float32), ws, wd], axis=1)

    w0aug = fold(W0, a_src0, a_dst0, N_HEAD, N_HID)        # [256, 520]
    w1aug = fold(W1, a_src1, a_dst1, N_HEAD, N_HID)        # [512, 520]
    w2aug = fold(W2, a_src2, a_dst2, N_HEAD_LAST, N_CLASS)  # [512, 252]

    # ---- edge routing ----
    core = dst // NLOC_REAL
    loc = dst - core * NLOC_REAL          # 0..1249
    tile_id = loc // 128
    dst_local = loc - tile_id * 128
    src_row = _row_of(_pid(src)).astype(np.int64)

    counts = np.zeros((NCORES, NTILE), dtype=np.int64)
    np.add.at(counts, (core, tile_id), 1)
    c_max = int(np.ceil(counts.max() / 128))
    S = c_max * 128

    # slot assignment per (core, tile); edges sorted by src row for DMA locality
    order = np.lexsort((src_row, tile_id, core))
    src_s, dl_s, core_s, tile_s = src_row[order], dst_local[order], core[order], tile_id[order]
    idx_all = np.zeros((NCORES, NTILE, 128, S // 16), dtype=np.int16)
    ind_all = np.zeros((NCORES, NTILE, 128, S), dtype=ml_dtypes.bfloat16)
    indt_all = np.zeros((NCORES, NTILE, 128, S), dtype=ml_dtypes.bfloat16)
    iota128 = np.arange(128, dtype=np.float32)
    pos = 0
    for k in range(NCORES):
        for t in range(NTILE):
            cnt = counts[k, t]
            sp = src_s[pos:pos + cnt]
            dl = dl_s[pos:pos + cnt]
            assert (core_s[pos:pos + cnt] == k).all() and (tile_s[pos:pos + cnt] == t).all()
            pos += cnt
            slots_src = np.zeros(S, dtype=np.int16)
            slots_src[:cnt] = sp.astype(np.int16)
            slots_dl = np.full(S, -1.0, dtype=np.float32)
            slots_dl[:cnt] = dl.astype(np.float32)
            if t == NTILE - 1:
                # give padded nodes (local 98..127 of the last tile) pad edges so
                # their softmax denominators stay finite (rows are dropped later)
                npads = S - cnt
                if npads > 0:
                    padnodes = 98 + (np.arange(npads) % 30)
                    slots_dl[cnt:] = padnodes.astype(np.float32)
            j = np.arange(S)
            idx_wrapped = np.zeros((16, S // 16), dtype=np.int16)
            idx_wrapped[j % 16, j // 16] = slots_src
            idx_all[k, t] = np.tile(idx_wrapped, (8, 1))
            sl2 = slots_dl.reshape(c_max, 128)           # [c, e]
            # Ind[e, c*128 + n] = (dst_local of slot (c,e)) == n
            m_e = (sl2[:, :, None] == iota128[None, None, :])   # [c, e, n]
            ind_all[k, t] = m_e.transpose(1, 0, 2).reshape(128, S).astype(ml_dtypes.bfloat16)
            # IndT[n, c*128 + e] = (dst_local of slot (c,e)) == n
            m_t = (sl2[None, :, :] == iota128[:, None, None])   # [n, c, e]
            indt_all[k, t] = m_t.reshape(128, S).astype(ml_dtypes.bfloat16)

    # ---- x transpose per core: [128, 2, 1280] (bf16) ----
    xpad = np.zeros((NPAD, N_FEAT), dtype=np.float32)
    xpad[_pid(np.arange(N))] = x
    xT = np.zeros((NCORES, 128, 2, NLOC), dtype=ml_dtypes.bfloat16)
    for k in range(NCORES):
        xl = xpad[k * NLOC:(k + 1) * NLOC]               # [1280, 256]
        xT[k] = xl.T.reshape(2, 128, NLOC).transpose(1, 0, 2).astype(ml_dtypes.bfloat16)

    def wlayout(W, kchunks, cols):
        # [in, cols] -> [128, kchunks, cols] (bf16)
        return np.ascontiguousarray(
            np.asarray(W, np.float32).reshape(kchunks, 128, cols).transpose(1, 0, 2)
        ).astype(ml_dtypes.bfloat16)

    # layer-2 slab layout: [xw 240 | p 6 | a_src 6 fp32]; reorder the aug
    # weights so a_src lands at ASRC2 on device (cols beyond D2+2*6 unused).
    common = {
        "w0aug": wlayout(w0aug, 2, D01 + 8),
        "wskip0": wlayout(np.asarray(Wskip_in, np.float32), 2, D01),
        "w1aug": wlayout(w1aug, 4, D01 + 8),
        "w2aug": wlayout(w2aug, 4, D2 + 12),
        "wskip2": wlayout(np.asarray(Wskip_out, np.float32), 4, N_CLASS),
        "b0row": np.asarray(b0, np.float32).reshape(1, D01),
        "b1row": np.asarray(b1, np.float32).reshape(1, D01),
        "b2row": np.asarray(b2, np.float32).reshape(1, N_CLASS),
        "ones_row": np.ones((1, 128), dtype=np.float32),
        "ident": np.eye(128, dtype=ml_dtypes.bfloat16),
    }
    in_maps = []
    for k in range(NCORES):
        m = dict(common)
        m["xT"] = xT[k]
        m["idx_all"] = idx_all[k].reshape(NTILE * 128, S // 16)
        m["ind_all"] = ind_all[k].reshape(NTILE * 128, S)
        m["indt_all"] = indt_all[k].reshape(NTILE * 128, S)
        in_maps.append(m)
    return c_max, in_maps


def kernel(**inputs):
    global LAST_RESULTS
    c_max, in_maps = _prep_inputs(**inputs)
    if c_max not in _PROGRAM_CACHE:
        _PROGRAM_CACHE[c_max] = _build_program(c_max)
    nc = _PROGRAM_CACHE[c_max]
    import os
    trace = bool(int(os.environ.get("GAT_TRACE", "0")))
    br = run_bass_kernel_spmd(nc, in_maps, list(range(NCORES)), trace=trace)
    LAST_RESULTS = br
    out = np.concatenate([np.asarray(r["out_loc"]) for r in br.results], axis=0)
    out = out.reshape(NCORES, NLOC, N_CLASS)[:, :NLOC_REAL].reshape(-1, N_CLASS)
    return np.ascontiguousarray(out[:N], dtype=np.float32)
